# revision 41
# baseline (speedup 1.0000x reference)
"""Trainium2 Bass kernel for nn_EncoderBlock (pre-norm self-attention + FFN).

Sharding (8 cores): core c -> batch b = c//4, head-group j = c%4
(heads 4j..4j+3, Wq columns 256j..256j+256, Wo rows 256j..256j+256),
token slice 256j..256j+256 for the post-attention residual + FFN part.
One ReduceScatter per q-half (the "all-reduce after Wo" of the TP hint,
fused with the token scatter) inside each 4-core batch group.

Key tricks (v2):
 - LN1 folded into the QKV matmuls via TWO augmented contraction rows
   (row0: -mu x g, row1: ones x c1 = Wq^T ln1_b), with the 1/std scale
   fused into the PSUM evacuation (single scalar_tensor_tensor op whose
   accum_out also yields colsum_v for free).
 - LN1 stats computed from a natural-layout copy of x on ACT (Square +
   accum_out) and GpSimd (free-axis reduce) -- zero PE work, overlapped
   with the QKV matmuls; 1/(std+eps) ~= Rsqrt(var) (error ~1e-6).
 - masked_fill(mask==0, 1e-9): exp runs directly on the scores PSUM
   (ACT, scale=1/8), then one fused bf16 op f = (e-1)*m at 2x DVE rate.
   The "masked entries contribute exactly 1" correction is a rank-1
   augmented row on the ctx matmul: ctx += colsum_v (x) ones, Z += S.
 - Softmax Z via an appended ones column (PSUM row 64); normalization
   uses a K=4 selector matmul to broadcast Z across partitions.
 - FFN token-sharded with full weights; W1 is prefetched once into SBUF
   (gpsimd DMA queue) and reused for both token blocks; ff1/relu for
   block 0 overlaps the second ReduceScatter; W2 streamed once.

All big matmuls run in bf16/float32r.
"""

import numpy as np
import ml_dtypes

import concourse.bass as bass
import concourse.mybir as mybir
import concourse.tile as tile
from concourse import bacc
from concourse import bass_utils
from concourse.masks import make_identity

F32 = mybir.dt.float32
F32R = mybir.dt.float32r
BF16 = mybir.dt.bfloat16
AF = mybir.ActivationFunctionType
MULT = mybir.AluOpType.mult
ADD = mybir.AluOpType.add
SUB = mybir.AluOpType.subtract
AX = mybir.AxisListType.X

B, S, D, H, DK, DFF = 2, 1024, 1024, 16, 64, 4096
EPS = 1e-6
P = 128
NC = 8
KS = D // P            # 8 k-subtiles over d_model
KA = KS + 1            # + augmented subtile (row0 = -mu, row1 = ones)
FFS = DFF // P         # 32 ff subtiles
FFA = FFS + 1          # + augmented subtile (row 0 = ones -> B2)
TS = S // 4            # 256-token slice per core
TM = TS // P           # 2 token M-tiles
SM = S // P            # 8 token M-tiles (full sequence)
HD = 4                 # heads per core
HCOLS = HD * DK        # 256 qkv columns per core
HM = HCOLS // P        # 2 partition strips of qkv columns
GROUPS = [[0, 1, 2, 3], [4, 5, 6, 7]]

_CACHE = {}


def _build():
    nc = bacc.Bacc("TRN2", target_bir_lowering=False, debug=False, num_devices=NC)

    def din(name, shape, dt):
        return nc.dram_tensor(name, shape, dt, kind="ExternalInput")

    xt = din("xt", [P, KA, S], BF16)          # x[b]^T striped + aug subtile
    xnat = din("xnat", [P, SM, D], BF16)      # x[b] natural, token-striped
    wq = din("wq", [P, KA, HCOLS], BF16)      # (Wq*a1) cols; aug row0=g row1=c1
    ones1 = din("ones1", [1, P], F32)        # ones row for partition-bcast
    caug = din("caug", [1, 68 * HD], F32)    # ctx aug lhsT: col 68h+64 = S
    xsl = din("xsl", [P, TM, D], BF16)        # x token-slice (natural)
    maskt = din("maskt", [P, KS, S], BF16)    # mask[b,0]^T striped
    wo = din("wo", [P, HM, D], BF16)          # Wo rows 256j..256j+256, striped
    w1 = din("w1", [P, KS, DFF], BF16)        # W1*a2 striped (lhsT)
    w2 = din("w2", [P, FFA, D], BF16)         # [W2 ; B2 ; 0-pad] striped (rhs)
    bias1 = din("bias1", [P, FFS], F32)       # B1 + W1^T ln2_b, striped cols
    out = nc.dram_tensor("out", [TS, D], F32, kind="ExternalOutput")

    with tile.TileContext(nc) as tc:
        with (
            tc.tile_pool(name="glob", bufs=1) as glob,
            tc.tile_pool(name="gdram", bufs=1, space="DRAM") as gdram,
        ):
            # ---- tiles that cross phases ----
            qkvT = glob.tile([P, HM, S], BF16)           # [d'(2 strips), tok]
            qkv = glob.tile([P, KS, 68 * HD], BF16)      # per-head 64+ones+3pad
            ctxn = glob.tile([P, HM, S], BF16)           # normalized ctx^T
            csum = glob.tile([P, HM], F32)               # colsum of qkvT strips
            w1f = glob.tile([P, KS, DFF], BF16)          # full W1, SBUF-resident
            w2f = glob.tile([P, FFA, D], BF16)           # full W2, SBUF-resident
            xslf = glob.tile([P, TM, D], BF16)           # x token-slice
            bias1t = glob.tile([P, FFS], F32)
            ones1t = glob.tile([1, P], F32)
            nc.sync.dma_start(ones1t[:], ones1[:])
            ones_s = glob.tile([1, S], F32)
            nc.gpsimd.memset(ones_s[:], 1.0)
            caugt = glob.tile([1, 68 * HD], F32)
            nc.sync.dma_start(caugt[:], caug[:])
            scr_mu = gdram.tile([SM, P], BF16)           # -mu round-trip
            scr_rs = gdram.tile([SM, P], F32)           # rsqrt(var) round-trip
            scr_cs = gdram.tile([HM, P], F32)           # colsum_v round-trip
            bounce_inA = gdram.tile([S // 2, D], BF16)   # attn-out partial, even
            bounce_inB = gdram.tile([S // 2, D], BF16)   # attn-out partial, odd
            bounce_rsA = gdram.tile([P, D], BF16)        # my tokens 0:128, summed
            bounce_rsB = gdram.tile([P, D], BF16)        # my tokens 128:256

            # ================= Phase A: LN1 stats + QKV =================
            with tc.tile_pool(name="pha", bufs=1) as pha:
                pha2_cm = tc.tile_pool(name="pha2", bufs=1)
                pha2 = pha2_cm.__enter__()
                psA_cm = tc.tile_pool(name="psA", bufs=1, space="PSUM")
                psA = psA_cm.__enter__()
                # All loads go on the SYNC ring in need-order (single FIFO
                # per ring => strict byte order, no starvation); all small
                # stores/round-trips go on the SCALAR ring. W1/W2 are
                # enqueued on the sync ring BEHIND every input via
                # tile_wait_until, then stream during attention.
                xtt = pha2.tile([P, KA, S], BF16)
                nc.sync.dma_start(xtt[:], xt[:])
                xnt_cs = []
                for c in range(2):
                    xc = pha2.tile([P, 2, D], BF16, tag="xnt", bufs=2)
                    nc.sync.dma_start(xc[:], xnat[:, 2 * c:2 * c + 2])
                    xnt_cs.append(xc)
                wqt = pha2.tile([P, KA, HCOLS], BF16)
                nc.sync.dma_start(wqt[:], wq[:])
                # mask loaded per q-half into one shared slot; half B goes
                # on the scalar ring so its slot-reuse stall (waits for
                # half-A exp consumption) never blocks the big loads
                maskA = pha.tile([P, KS, S // 2], BF16, tag="mask", bufs=1)
                nc.sync.dma_start(maskA[:], maskt[:, :, 0:512])
                wot = pha.tile([P, HM, D], BF16)
                nc.sync.dma_start(wot[:], wo[:])
                nc.sync.dma_start(xslf[:], xsl[:])
                nc.sync.dma_start(bias1t[:], bias1[:])
                # chunks 2-3 reuse slots -> their ring entries stall until
                # the stats consume chunks 0-1 (~t=20us); keep them last
                # so nothing queues behind the stall
                for c in range(2, 4):
                    xc = pha2.tile([P, 2, D], BF16, tag="xnt", bufs=2)
                    nc.sync.dma_start(xc[:], xnat[:, 2 * c:2 * c + 2])
                    xnt_cs.append(xc)
                # LN1 stats: S2 on ACT (Square + accum), S1 on DVE
                s1c = pha2.tile([P, SM], F32)
                s2c = pha2.tile([P, SM], F32)
                sqscr = pha2.tile([P, D], BF16, tag="sqscr", bufs=1)
                cpscr = pha2.tile([P, D], BF16, tag="cpscr", bufs=1)
                for t in range(SM):
                    xsrc = xnt_cs[t // 2][:, t % 2]
                    nc.scalar.activation(
                        sqscr[:], xsrc, AF.Square,
                        accum_out=s2c[:, t:t + 1],
                    )
                    nc.vector.tensor_scalar(
                        out=cpscr[:], in0=xsrc, scalar1=1.0,
                        scalar2=0.0, op0=MULT, op1=ADD,
                        accum_out=s1c[:, t:t + 1],
                    )
                # column math on [P, SM]: mu, var, rsqrt(var)
                mu_c = pha2.tile([P, SM], F32)
                nc.vector.tensor_scalar_mul(mu_c[:], s1c[:], 1.0 / D)
                tv = pha2.tile([P, SM], F32)
                nc.vector.tensor_tensor(tv[:], s1c[:], mu_c[:], MULT)
                nc.vector.tensor_tensor(tv[:], s2c[:], tv[:], SUB)
                nc.vector.tensor_scalar_mul(tv[:], tv[:], 1.0 / (D - 1))
                stdc = pha2.tile([P, SM], F32)
                nc.scalar.activation(stdc[:], tv[:], AF.Sqrt)
                nc.vector.tensor_scalar_add(stdc[:], stdc[:], EPS)
                rsq_c = pha2.tile([P, SM], F32)           # = r1col (recip std)
                nc.vector.reciprocal(rsq_c[:], stdc[:])
                mu_n = pha2.tile([P, SM], BF16)
                nc.vector.tensor_scalar_mul(mu_n[:], s1c[:], -1.0 / D)

                # round-trips to row layout: -mu -> xt aug row0, rsq -> r1row
                nc.scalar.dma_start(
                    scr_mu[:].rearrange("t p -> p t"), mu_n[:]
                )
                nc.scalar.dma_start(
                    xtt[0:1, KS, :],
                    scr_mu[:].rearrange("t p -> (t p)").unsqueeze(0),
                )
                nc.scalar.dma_start(
                    scr_rs[:].rearrange("t p -> p t"), rsq_c[:]
                )
                r1row = pha2.tile([1, S], F32)
                nc.scalar.dma_start(
                    r1row[:],
                    scr_rs[:].rearrange("t p -> (t p)").unsqueeze(0),
                )

                # PE program order: the k<=7 accumulations of both qkvT
                # strips and the first two qkv-natural tiles gate only on
                # xtt/wqt, so the PE starts ~6us in; the r1 broadcast, aug
                # matmuls and evacs (gated on the stats chain) come after.
                ps_qts = [
                    psA.tile([P, S], F32, name=f"ps_qt{m}", bufs=1)
                    for m in range(HM)
                ]
                for m in range(HM):
                    for nb in range(2):
                        qs = slice(512 * nb, 512 * nb + 512)
                        for k in range(KS):
                            nc.tensor.matmul(
                                ps_qts[m][:, qs],
                                wqt[:, k, m * P:(m + 1) * P],
                                xtt[:, k, qs],
                                start=(k == 0), stop=False,
                            )
                for h in range(HD):
                    nc.gpsimd.memset(qkv[:, :, 68 * h + 64:68 * h + 65], 1.0)

                def qkv_nat_main(m):
                    ps_q = psA.tile([P, HCOLS], F32, name=f"ps_q{m % 2}",
                                    bufs=1)
                    for k in range(KS):
                        nc.tensor.matmul(
                            ps_q[:], xtt[:, k, m * P:(m + 1) * P], wqt[:, k],
                            start=(k == 0), stop=False,
                        )
                    return ps_q

                def qkv_nat_fin(m, ps_q):
                    nc.tensor.matmul(
                        ps_q[:], xtt[:, KS, m * P:(m + 1) * P], wqt[:, KS],
                        start=False, stop=True,
                    )
                    nc.vector.tensor_scalar_mul(
                        qkv[:, m, :].rearrange(
                            "p (h c) -> p h c", c=68
                        )[:, :, 0:64],
                        ps_q[:].rearrange("p (h c) -> p h c", c=64),
                        rsq_c[:, m:m + 1],
                    )

                early = {m: qkv_nat_main(m) for m in range(2)}

                # broadcast r1 across partitions via K=1 matmul
                ps_r1 = psA.tile([P, S], F32, name="ps_r1", tag="ps_a")
                for nb in range(2):
                    qs = slice(512 * nb, 512 * nb + 512)
                    nc.tensor.matmul(
                        ps_r1[:, qs], ones1t[:], r1row[:, qs],
                        start=True, stop=True,
                    )
                R1 = pha.tile([P, S], BF16, tag="bcastbuf", bufs=2)
                nc.vector.tensor_copy(R1[:], ps_r1[:])

                # close the strip groups with the aug subtile, then evac
                # qkvT = psum * r1 (accum_out gives colsum_v for free)
                for m in range(HM):
                    for nb in range(2):
                        qs = slice(512 * nb, 512 * nb + 512)
                        nc.tensor.matmul(
                            ps_qts[m][:, qs],
                            wqt[:, KS, m * P:(m + 1) * P],
                            xtt[:, KS, qs],
                            start=False, stop=True,
                        )
                    nc.vector.scalar_tensor_tensor(
                        out=qkvT[:, m, :], in0=ps_qts[m][:], scalar=1.0,
                        in1=R1[:], op0=MULT, op1=MULT,
                        accum_out=csum[:, m:m + 1],
                    )

                # colsum_v -> row layout -> caugt head segments
                nc.scalar.dma_start(
                    scr_cs[:].rearrange("t p -> p t"), csum[:]
                )
                nc.scalar.dma_start(
                    caugt[0:1, :].rearrange(
                        "one (h c) -> one h c", c=68
                    )[:, :, 0:64],
                    scr_cs[:].rearrange("t p -> (t p)").unsqueeze(0)
                    .rearrange("one (h c) -> one h c", c=64),
                )

                # W1/W2 prefetch on the SWDGE (gpsimd) generator (a large
                # HWDGE transfer would block both hw rings' descriptor
                # generation for its whole duration). Emitted AFTER every
                # latency-critical round-trip DMA: Tile assigns the 8 DMA
                # completion semaphore lanes round-robin in emission
                # order, and a round trip emitted after the weights can
                # alias onto a lane whose wait target includes the 16.6MB
                # transfers -- its consumer then stalls until the weights
                # finish (~85us, measured). Runtime start is still gated
                # to ~inputs-done via a real write-after-write hazard: a
                # tiny copy (reads bias1t, the last small load; writes one
                # element of the destination) ordered before each DMA.
                for wdst, wsrc in ((w1f, w1), (w2f, w2)):
                    nc.vector.tensor_copy(wdst[0:1, 0, 0:1],
                                          bias1t[0:1, 0:1])
                    nc.gpsimd.dma_start(wdst[:], wsrc[:])

                for m in range(2):
                    qkv_nat_fin(m, early[m])
                for m in range(2, KS):
                    ps_q = qkv_nat_main(m)
                    qkv_nat_fin(m, ps_q)

                psA_cm.__exit__(None, None, None)
                pha2_cm.__exit__(None, None, None)

                # identity for phase-D transposes, built BEFORE the
                # collectives are issued (gpsimd queue would otherwise
                # stall it behind the ReduceScatters)
                ident = glob.tile([P, P], BF16)
                make_identity(nc, ident[:])

                # ============= Phase B/C: attention pipelined by q-half ====
                with (
                    tc.tile_pool(name="phb", bufs=1) as phb,
                    tc.tile_pool(name="psB", bufs=1, space="PSUM") as psB,
                ):
                    for qh in range(2):
                        qsl = slice(512 * qh, 512 * qh + 512)
                        if qh == 0:
                            maskq = maskA
                        else:
                            # sync ring: empty after the inputs (weights
                            # are SWDGE), so maskB's slot-reuse stall
                            # blocks nothing; the scalar ring must stay
                            # clear for the bounce writes -> earlier RS
                            maskq = pha.tile([P, KS, S // 2], BF16,
                                             tag="mask", bufs=1)
                            nc.sync.dma_start(maskq[:],
                                              maskt[:, :, 512:1024])
                        for pair in range(2):
                            # two m-subtiles per exp/fixup op; ctx matmuls
                            # interleave with scores so the PE never idles
                            # long enough for HAM to re-throttle
                            pcts = {}
                            for h in (2 * pair, 2 * pair + 1):
                                pcts[h] = psB.tile(
                                    [P, 512], F32, name=f"ps_ct{h % 2}",
                                    bufs=1,
                                )
                            for mm2 in range(4):
                                eTms = {}
                                for h in (2 * pair, 2 * pair + 1):
                                    hp = 64 * (h % 2)
                                    hs = h // 2
                                    ps_sc = psB.tile(
                                        [P, 1024], F32, name="ps_sc", bufs=2
                                    )
                                    for half in range(2):
                                        m = 2 * mm2 + half
                                        nc.tensor.matmul(
                                            ps_sc[:, 512 * half:
                                                  512 * half + 512],
                                            qkvT[hp:hp + 64, hs,
                                                 m * P:(m + 1) * P],
                                            qkvT[hp:hp + 64, hs, qsl],
                                            start=True, stop=True,
                                        )
                                    eraw = phb.tile(
                                        [P, 1024], BF16, name="eraw", bufs=4
                                    )
                                    nc.scalar.activation(
                                        eraw[:], ps_sc[:], AF.Exp,
                                        scale=float(1.0 / np.sqrt(DK)),
                                    )
                                    eTm = phb.tile(
                                        [P, 1024], BF16, name="eTm", bufs=4
                                    )
                                    eTms[h] = eTm
                                    mk = maskq[:, 2 * mm2:2 * mm2 + 2, :]
                                    em1 = phb.tile(
                                        [P, 1024], BF16, name="em1", bufs=4
                                    )
                                    nc.vector.tensor_scalar_sub(
                                        em1[:], eraw[:], 1.0
                                    )
                                    nc.vector.tensor_tensor(
                                        eTm[:], em1[:], mk, MULT
                                    )
                                for h in (2 * pair, 2 * pair + 1):
                                    for half in range(2):
                                        nc.tensor.matmul(
                                            pcts[h][0:65, :],
                                            qkv[:, 2 * mm2 + half,
                                                68 * h:68 * h + 65],
                                            eTms[h][:, 512 * half:
                                                    512 * half + 512],
                                            start=(mm2 == 0 and half == 0),
                                            stop=False,
                                        )
                            zhs = {}
                            for h in (2 * pair, 2 * pair + 1):
                                nc.tensor.matmul(
                                    pcts[h][0:65, :],
                                    caugt[0:1, 68 * h:68 * h + 65],
                                    ones_s[0:1, qsl],
                                    start=False, stop=True,
                                )
                                zh = phb.tile([1, 512], F32,
                                              name=f"zh{h % 2}",
                                              tag="zh", bufs=4)
                                zhs[h] = zh
                                nc.vector.tensor_copy(
                                    zh[:], pcts[h][64:65, :]
                                )
                            ps_nb = psB.tile([P, 1024], F32, name="ps_sc",
                                             bufs=2)
                            for h in (2 * pair, 2 * pair + 1):
                                hp = 64 * (h % 2)
                                nc.tensor.matmul(
                                    ps_nb[hp:hp + 64, 0:512],
                                    ones1t[0:1, 0:64], zhs[h][:],
                                    start=True, stop=True,
                                )
                            nrmb = pha.tile([P, 512], F32,
                                            name=f"nrmb{pair}_{qh}",
                                            tag="bcastbuf", bufs=2)
                            nc.vector.reciprocal(nrmb[:], ps_nb[:, 0:512])
                            for h in (2 * pair, 2 * pair + 1):
                                hp = 64 * (h % 2)
                                nc.vector.scalar_tensor_tensor(
                                    out=ctxn[hp:hp + 64, pair, qsl],
                                    in0=pcts[h][0:64, :], scalar=1.0,
                                    in1=nrmb[hp:hp + 64, :],
                                    op0=MULT, op1=MULT,
                                )
                        # attn-out partial for this q-half, then its RS
                        bnc = (bounce_inA if qh == 0 else bounce_inB)[
                            :
                        ].rearrange("(r p) d -> p r d", p=P)
                        for mt in range(4 * qh, 4 * qh + 4):
                            ps_ao = psB.tile([P, D], F32, name="ps_ao",
                                             bufs=1)
                            for nb in range(2):
                                ds_ = slice(512 * nb, 512 * nb + 512)
                                for k in range(HM):
                                    nc.tensor.matmul(
                                        ps_ao[:, ds_],
                                        ctxn[:, k, mt * P:(mt + 1) * P],
                                        wot[:, k, ds_],
                                        start=(k == 0), stop=(k == HM - 1),
                                    )
                            aot = phb.tile([P, D], BF16, tag="aot", bufs=2)
                            nc.scalar.copy(aot[:], ps_ao[:])
                            nc.sync.dma_start(bnc[:, mt - 4 * qh, :],
                                              aot[:])

            nc.gpsimd.collective_compute(
                "ReduceScatter",
                mybir.AluOpType.add,
                replica_groups=GROUPS,
                ins=[bounce_inA.opt()],
                outs=[bounce_rsA.opt()],
            )
            nc.gpsimd.collective_compute(
                "ReduceScatter",
                mybir.AluOpType.add,
                replica_groups=GROUPS,
                ins=[bounce_inB.opt()],
                outs=[bounce_rsB.opt()],
            )

            # ========== Phase D: residual + LN2 + FFN ==========
            with (
                tc.tile_pool(name="phd", bufs=1) as phd,
                tc.tile_pool(name="psD", bufs=1, space="PSUM") as psD,
            ):
                x2s = [None, None]
                n2s = [None, None]
                n2Ts = [None, None]

                def ln2_block(m):
                    aors = phd.tile([P, D], BF16, name=f"aors{m}")
                    nc.sync.dma_start(
                        aors[:], (bounce_rsA if m == 0 else bounce_rsB)[:]
                    )
                    x2 = phd.tile([P, D], F32, name=f"x2_{m}")
                    nc.vector.tensor_tensor(x2[:], aors[:], xslf[:, m], ADD)

                    s1 = phd.tile([P, 1], F32, name=f"s1_{m}")
                    s2 = phd.tile([P, 1], F32, name=f"s2_{m}")
                    sq2 = phd.tile([P, D], F32, tag="sq2", bufs=1)
                    nc.vector.reduce_sum(out=s1[:], in_=x2[:], axis=AX)
                    nc.scalar.activation(
                        sq2[:], x2[:], AF.Square, accum_out=s2[:]
                    )
                    mu2 = phd.tile([P, 1], F32, name=f"mu2_{m}")
                    nc.vector.tensor_scalar_mul(mu2[:], s1[:], 1.0 / D)
                    v2 = phd.tile([P, 1], F32, name=f"v2_{m}")
                    nc.vector.tensor_tensor(v2[:], s1[:], mu2[:], MULT)
                    nc.vector.tensor_tensor(v2[:], s2[:], v2[:], SUB)
                    nc.vector.tensor_scalar_mul(v2[:], v2[:], 1.0 / (D - 1))
                    std2 = phd.tile([P, 1], F32, name=f"std2_{m}")
                    nc.scalar.activation(std2[:], v2[:], AF.Sqrt)
                    nc.vector.tensor_scalar_add(std2[:], std2[:], EPS)
                    r2 = phd.tile([P, 1], F32, name=f"r2_{m}")
                    nc.vector.reciprocal(r2[:], std2[:])
                    n2 = phd.tile([P, D], BF16, name=f"n2_{m}")
                    nc.vector.scalar_tensor_tensor(
                        out=n2[:], in0=x2[:], scalar=mu2[:], op0=SUB,
                        in1=r2[:].to_broadcast((P, D)), op1=MULT,
                    )
                    x2s[m] = x2
                    n2s[m] = n2

                def transp_block(m):
                    n2T = phd.tile([P, KS, P], BF16, name=f"n2T{m}")
                    for i in range(KS):
                        ps_t = psD.tile([P, P], BF16, name="ps_t", bufs=2)
                        nc.tensor.transpose(
                            ps_t[:], n2s[m][:, i * P:(i + 1) * P], ident[:]
                        )
                        nc.vector.tensor_copy(n2T[:, i, :], ps_t[:])
                    n2Ts[m] = n2T

                def ff1_block(m, bias1t, reluT):
                    for g in range(8):
                        for i4 in range(4):
                            i = 4 * g + i4
                            ps_f = psD.tile([P, P], F32, name="ps_f", bufs=2)
                            for k in range(KS):
                                nc.tensor.matmul(
                                    ps_f[:],
                                    w1f[:, k, 128 * i:128 * i + 128],
                                    n2Ts[m][:, k, :],
                                    start=(k == 0), stop=(k == KS - 1),
                                )
                            nc.scalar.activation(
                                reluT[:, i, m * P:(m + 1) * P], ps_f[:],
                                AF.Relu, bias=bias1t[:, i:i + 1],
                            )

                # m=0 chain first (overlaps the second ReduceScatter);
                # every DMA that m=0 needs is issued before the aors1 DMA
                # so the RS-B wait cannot head-of-line-block them.
                ps_o0 = psD.tile([P, D], F32, name="ps_o0")
                ps_o1 = psD.tile([P, D], F32, name="ps_o1")
                ps_os = [ps_o0, ps_o1]

                def ff2_block(m, reluT):
                    for k in range(FFA):
                        for nb in range(2):
                            ds_ = slice(512 * nb, 512 * nb + 512)
                            nc.tensor.matmul(
                                ps_os[m][:, ds_],
                                reluT[:, k, m * P:(m + 1) * P],
                                w2f[:, k, ds_],
                                start=(k == 0), stop=(k == FFA - 1),
                            )
                    outt = phd.tile([P, D], F32, name=f"outt{m}")
                    nc.vector.tensor_tensor(
                        outt[:], ps_os[m][:], x2s[m][:], ADD
                    )
                    nc.scalar.dma_start(
                        out[:].rearrange("(m p) d -> p m d", p=P)[:, m, :],
                        outt[:],
                    )

                ln2_block(0)
                # memsets on DVE: the gpsimd queue is blocked behind the
                # collective triggers at this point in the program
                reluT = phd.tile([P, FFA, TS], BF16)
                nc.vector.memset(reluT[:, FFS, :], 0.0)
                nc.vector.memset(reluT[0:1, FFS, :], 1.0)
                transp_block(0)
                ff1_block(0, bias1t, reluT)
                ff2_block(0, reluT)
                ln2_block(1)
                transp_block(1)
                ff1_block(1, bias1t, reluT)
                ff2_block(1, reluT)
    nc.compile()
    return nc


def _prep_inputs(x, mask, Wq, Wo, W1, B1, W2, B2, ln1_a, ln1_b, ln2_a, ln2_b):
    """Host-side folding + striping. Returns in_maps for 8 cores."""
    f32 = np.float32

    def strip(a, ks):  # [ks*128, F] -> [128, ks, F]
        return np.ascontiguousarray(
            a.reshape(ks, P, -1).transpose(1, 0, 2).astype(f32)
        )

    Wa = (Wq * ln1_a[:, None]).astype(f32)          # LN1 scale folded
    g = Wa.sum(axis=0)                               # [D]
    c1 = (Wq.T @ ln1_b).astype(f32)                  # [D]
    Wa1 = (W1 * ln2_a[:, None]).astype(f32)
    bias1_full = (B1 + W1.T @ ln2_b).astype(f32)     # [DFF]

    w1_s = strip(Wa1, KS).astype(ml_dtypes.bfloat16)  # [128, 8, 4096]
    w2_aug = np.zeros((FFA * P, D), f32)
    w2_aug[:DFF] = W2
    w2_aug[DFF] = B2
    w2_s = strip(w2_aug, FFA).astype(ml_dtypes.bfloat16)  # [128, 33, 1024]
    bias1_s = np.ascontiguousarray(bias1_full.reshape(FFS, P).T)  # [128, 32]

    ones1 = np.ones((1, P), f32)
    caug_h = np.zeros((1, 68 * HD), f32)
    for h in range(HD):
        caug_h[0, 68 * h + 64] = float(S)

    in_maps = []
    for c in range(NC):
        b, j = divmod(c, 4)
        cols = slice(HCOLS * j, HCOLS * j + HCOLS)
        tok_blocks = [j, j + 4]  # 128-token blocks owned by this core

        xt_aug = np.zeros((P, KA, S), ml_dtypes.bfloat16)
        xt_aug[:, :KS] = strip(np.ascontiguousarray(x[b].T), KS)
        xt_aug[1, KS] = 1.0
        xnat_s = strip(np.asarray(x[b], f32), SM).astype(ml_dtypes.bfloat16)
        wq_aug = np.zeros((P, KA, HCOLS), ml_dtypes.bfloat16)
        wq_aug[:, :KS] = strip(Wa[:, cols], KS)
        wq_aug[0, KS] = g[cols]
        wq_aug[1, KS] = c1[cols]
        maskt_ = np.ascontiguousarray(mask[b, 0].T).astype(f32)

        in_maps.append({
            "xt": xt_aug,
            "xnat": xnat_s,
            "wq": wq_aug,
            "ones1": ones1,
            "caug": caug_h,
            "xsl": np.ascontiguousarray(np.stack(
                [x[b, 128 * t:128 * t + 128] for t in tok_blocks], axis=1
            ).transpose(0, 1, 2)).reshape(P, TM, D).astype(ml_dtypes.bfloat16),
            "maskt": strip(maskt_, KS).astype(ml_dtypes.bfloat16),
            "wo": strip(
                np.ascontiguousarray(np.asarray(Wo, f32)[cols]), HM
            ).astype(ml_dtypes.bfloat16),
            "w1": w1_s,
            "w2": w2_s,
            "bias1": bias1_s,
        })
    return in_maps


def kernel(**inputs):
    if "nc" not in _CACHE:
        _CACHE["nc"] = _build()
    nc = _CACHE["nc"]
    args = {k: np.asarray(v) for k, v in inputs.items()}
    in_maps = _prep_inputs(
        args["x"], args["mask"], args["Wq"], args["Wo"], args["W1"],
        args["B1"], args["W2"], args["B2"], args["ln1_a"], args["ln1_b"],
        args["ln2_a"], args["ln2_b"],
    )
    res = bass_utils.run_bass_kernel_spmd(
        nc, in_maps, core_ids=list(range(NC))
    )
    out = np.empty((B, S, D), np.float32)
    for c in range(NC):
        b, j = divmod(c, 4)
        o = res.results[c]["out"]
        out[b, 128 * j:128 * j + 128] = o[0:128]
        out[b, 512 + 128 * j:512 + 128 * j + 128] = o[128:256]
    return out



# revision 45
# speedup vs baseline: 1.0190x; 1.0190x over previous
"""Trainium2 Bass kernel for nn_EncoderBlock (pre-norm self-attention + FFN).

Sharding (8 cores): core c -> batch b = c//4, head-group j = c%4
(heads 4j..4j+3, Wq columns 256j..256j+256, Wo rows 256j..256j+256),
token slice 256j..256j+256 for the post-attention residual + FFN part.
One ReduceScatter per q-half (the "all-reduce after Wo" of the TP hint,
fused with the token scatter) inside each 4-core batch group.

Key tricks (v2):
 - LN1 folded into the QKV matmuls via TWO augmented contraction rows
   (row0: -mu x g, row1: ones x c1 = Wq^T ln1_b), with the 1/std scale
   fused into the PSUM evacuation (single scalar_tensor_tensor op whose
   accum_out also yields colsum_v for free).
 - LN1 stats computed from a natural-layout copy of x on ACT (Square +
   accum_out) and GpSimd (free-axis reduce) -- zero PE work, overlapped
   with the QKV matmuls; 1/(std+eps) ~= Rsqrt(var) (error ~1e-6).
 - masked_fill(mask==0, 1e-9): exp runs directly on the scores PSUM
   (ACT, scale=1/8), then one fused bf16 op f = (e-1)*m at 2x DVE rate.
   The "masked entries contribute exactly 1" correction is a rank-1
   augmented row on the ctx matmul: ctx += colsum_v (x) ones, Z += S.
 - Softmax Z via an appended ones column (PSUM row 64); normalization
   uses a K=4 selector matmul to broadcast Z across partitions.
 - FFN token-sharded with full weights; W1 is prefetched once into SBUF
   (gpsimd DMA queue) and reused for both token blocks; ff1/relu for
   block 0 overlaps the second ReduceScatter; W2 streamed once.

All big matmuls run in bf16/float32r.
"""

import numpy as np
import ml_dtypes

import concourse.bass as bass
import concourse.mybir as mybir
import concourse.tile as tile
from concourse import bacc
from concourse import bass_utils
from concourse.masks import make_identity

F32 = mybir.dt.float32
F32R = mybir.dt.float32r
BF16 = mybir.dt.bfloat16
AF = mybir.ActivationFunctionType
MULT = mybir.AluOpType.mult
ADD = mybir.AluOpType.add
SUB = mybir.AluOpType.subtract
AX = mybir.AxisListType.X

B, S, D, H, DK, DFF = 2, 1024, 1024, 16, 64, 4096
EPS = 1e-6
P = 128
NC = 8
KS = D // P            # 8 k-subtiles over d_model
KA = KS + 1            # + augmented subtile (row0 = -mu, row1 = ones)
FFS = DFF // P         # 32 ff subtiles
FFA = FFS + 1          # + augmented subtile (row 0 = ones -> B2)
TS = S // 4            # 256-token slice per core
TM = TS // P           # 2 token M-tiles
SM = S // P            # 8 token M-tiles (full sequence)
HD = 4                 # heads per core
HCOLS = HD * DK        # 256 qkv columns per core
HM = HCOLS // P        # 2 partition strips of qkv columns
GROUPS = [[0, 1, 2, 3], [4, 5, 6, 7]]

_CACHE = {}


def _build():
    nc = bacc.Bacc("TRN2", target_bir_lowering=False, debug=False, num_devices=NC)

    def din(name, shape, dt):
        return nc.dram_tensor(name, shape, dt, kind="ExternalInput")

    xt = din("xt", [P, KA, S], BF16)          # x[b]^T striped + aug subtile
    xnat = din("xnat", [P, SM, D], BF16)      # x[b] natural, token-striped
    wq = din("wq", [P, KA, HCOLS], BF16)      # (Wq*a1) cols; aug row0=g row1=c1
    ones1 = din("ones1", [1, P], F32)        # ones row for partition-bcast
    caug = din("caug", [1, 68 * HD], BF16)    # ctx aug lhsT: col 68h+64 = S
    xsl = din("xsl", [P, TM, D], BF16)        # x token-slice (natural)
    maskt = din("maskt", [P, KS, S], BF16)    # mask[b,0]^T striped
    wo = din("wo", [P, HM, D], BF16)          # Wo rows 256j..256j+256, striped
    w1 = din("w1", [P, KS, DFF], BF16)        # W1*a2 striped (lhsT)
    w2 = din("w2", [P, FFA, D], BF16)         # [W2 ; B2 ; 0-pad] striped (rhs)
    bias1 = din("bias1", [P, FFS], F32)       # B1 + W1^T ln2_b, striped cols
    out = nc.dram_tensor("out", [TS, D], F32, kind="ExternalOutput")

    with tile.TileContext(nc) as tc:
        with (
            tc.tile_pool(name="glob", bufs=1) as glob,
            tc.tile_pool(name="gdram", bufs=1, space="DRAM") as gdram,
        ):
            # ---- tiles that cross phases ----
            qkvT = glob.tile([P, HM, S], BF16)           # [d'(2 strips), tok]
            qkv = glob.tile([P, KS, 68 * HD], BF16)      # per-head 64+ones+3pad
            ctxn = glob.tile([P, HM, S], BF16)           # normalized ctx^T
            csum = glob.tile([P, HM], BF16)               # colsum of qkvT strips
            w1f = glob.tile([P, KS, DFF], BF16)          # full W1, SBUF-resident
            w2f = glob.tile([P, FFA, D], BF16)           # full W2, SBUF-resident
            xslf = glob.tile([P, TM, D], BF16)           # x token-slice
            bias1t = glob.tile([P, FFS], F32)
            ones1t = glob.tile([1, P], F32)
            nc.sync.dma_start(ones1t[:], ones1[:])
            ones_s = glob.tile([1, S], BF16)
            nc.gpsimd.memset(ones_s[:], 1.0)
            caugt = glob.tile([1, 68 * HD], BF16)
            nc.sync.dma_start(caugt[:], caug[:])
            scr_mu = gdram.tile([SM, P], BF16)           # -mu round-trip
            scr_rs = gdram.tile([SM, P], F32)           # rsqrt(var) round-trip
            scr_cs = gdram.tile([HM, P], BF16)           # colsum_v round-trip
            bounce_inA = gdram.tile([S // 2, D], BF16)   # attn-out partial, even
            bounce_inB = gdram.tile([S // 2, D], BF16)   # attn-out partial, odd
            bounce_rsA = gdram.tile([P, D], BF16)        # my tokens 0:128, summed
            bounce_rsB = gdram.tile([P, D], BF16)        # my tokens 128:256

            # ================= Phase A: LN1 stats + QKV =================
            with tc.tile_pool(name="pha", bufs=1) as pha:
                pha2_cm = tc.tile_pool(name="pha2", bufs=1)
                pha2 = pha2_cm.__enter__()
                psA_cm = tc.tile_pool(name="psA", bufs=1, space="PSUM")
                psA = psA_cm.__enter__()
                # All loads go on the SYNC ring in need-order (single FIFO
                # per ring => strict byte order, no starvation); all small
                # stores/round-trips go on the SCALAR ring. W1/W2 are
                # enqueued on the sync ring BEHIND every input via
                # tile_wait_until, then stream during attention.
                xtt = pha2.tile([P, KA, S], BF16)
                nc.sync.dma_start(xtt[:], xt[:])
                xnt_cs = []
                for c in range(1):
                    xc = pha2.tile([P, 2, D], BF16, tag="xnt", bufs=1)
                    nc.sync.dma_start(xc[:], xnat[:, 2 * c:2 * c + 2])
                    xnt_cs.append(xc)
                wqt = pha2.tile([P, KA, HCOLS], BF16)
                nc.sync.dma_start(wqt[:], wq[:])
                # both mask halves resident up front (two slots): the
                # single-slot variant serializes qh1 behind the LAST qh0
                # exp consumer mid-attention (~20us, measured)
                maskA = pha.tile([P, KS, S // 2], BF16, tag="mask", bufs=2)
                nc.sync.dma_start(maskA[:], maskt[:, :, 0:512])
                maskB2 = pha.tile([P, KS, S // 2], BF16, tag="mask", bufs=2)
                nc.sync.dma_start(maskB2[:], maskt[:, :, 512:1024])
                wot = pha.tile([P, HM, D], BF16)
                nc.sync.dma_start(wot[:], wo[:])
                nc.sync.dma_start(xslf[:], xsl[:])
                nc.sync.dma_start(bias1t[:], bias1[:])
                # chunks 1-3 reuse the slot -> their ring entries stall
                # until the stats consume the prior chunk; keep them last
                # so nothing queues behind the stall
                for c in range(1, 4):
                    xc = pha2.tile([P, 2, D], BF16, tag="xnt", bufs=1)
                    nc.sync.dma_start(xc[:], xnat[:, 2 * c:2 * c + 2])
                    xnt_cs.append(xc)
                # W1/W2 prefetch on the SWDGE (gpsimd) generator: a large
                # HWDGE transfer blocks BOTH hw rings' descriptor
                # generation for its whole duration, which would stall the
                # small latency-critical stats round-trips behind it.
                # Gate each prefetch behind the inputs with a REAL
                # write-after-write hazard: a tiny copy (reads bias1t, the
                # last small load; writes one element of the destination)
                # that the overwriting DMA must order after.
                for wdst, wsrc in ((w1f, w1), (w2f, w2)):
                    nc.vector.tensor_copy(wdst[0:1, 0, 0:1],
                                          bias1t[0:1, 0:1])
                    nc.gpsimd.dma_start(wdst[:], wsrc[:])

                # LN1 stats: S2 on ACT (Square + accum), S1 on DVE
                s1c = pha2.tile([P, SM], F32)
                s2c = pha2.tile([P, SM], F32)
                sqscr = pha2.tile([P, D], BF16, tag="sqscr", bufs=1)
                for t in range(SM):
                    xsrc = xnt_cs[t // 2][:, t % 2]
                    nc.scalar.activation(
                        sqscr[:], xsrc, AF.Square,
                        accum_out=s2c[:, t:t + 1],
                    )
                    nc.vector.reduce_sum(
                        out=s1c[:, t:t + 1], in_=xsrc, axis=AX,
                    )
                # column math on [P, SM]: mu, var, rsqrt(var)
                mu_c = pha2.tile([P, SM], F32)
                nc.vector.tensor_scalar_mul(mu_c[:], s1c[:], 1.0 / D)
                tv = pha2.tile([P, SM], F32)
                nc.vector.tensor_tensor(tv[:], s1c[:], mu_c[:], MULT)
                nc.vector.tensor_tensor(tv[:], s2c[:], tv[:], SUB)
                nc.vector.tensor_scalar_mul(tv[:], tv[:], 1.0 / (D - 1))
                stdc = pha2.tile([P, SM], F32)
                nc.scalar.activation(stdc[:], tv[:], AF.Sqrt)
                nc.vector.tensor_scalar_add(stdc[:], stdc[:], EPS)
                rsq_c = pha2.tile([P, SM], F32)           # = r1col (recip std)
                nc.vector.reciprocal(rsq_c[:], stdc[:])
                mu_n = pha2.tile([P, SM], BF16)
                nc.vector.tensor_scalar_mul(mu_n[:], s1c[:], -1.0 / D)

                # round-trips to row layout: -mu -> xt aug row0, rsq -> r1row
                nc.scalar.dma_start(
                    scr_mu[:].rearrange("t p -> p t"), mu_n[:]
                )
                nc.scalar.dma_start(
                    xtt[0:1, KS, :],
                    scr_mu[:].rearrange("t p -> (t p)").unsqueeze(0),
                )
                nc.scalar.dma_start(
                    scr_rs[:].rearrange("t p -> p t"), rsq_c[:]
                )
                r1row = pha2.tile([1, S], F32)
                nc.scalar.dma_start(
                    r1row[:],
                    scr_rs[:].rearrange("t p -> (t p)").unsqueeze(0),
                )

                # PE program order: the k<=7 accumulations of both qkvT
                # strips and the first two qkv-natural tiles gate only on
                # xtt/wqt, so the PE starts ~6us in; the r1 broadcast, aug
                # matmuls and evacs (gated on the stats chain) come after.
                ps_qts = [
                    psA.tile([P, S], F32, name=f"ps_qt{m}", bufs=1)
                    for m in range(HM)
                ]
                for m in range(HM):
                    for nb in range(2):
                        qs = slice(512 * nb, 512 * nb + 512)
                        for k in range(KS):
                            nc.tensor.matmul(
                                ps_qts[m][:, qs],
                                wqt[:, k, m * P:(m + 1) * P],
                                xtt[:, k, qs],
                                start=(k == 0), stop=False,
                            )
                for h in range(HD):
                    nc.gpsimd.memset(qkv[:, :, 68 * h + 64:68 * h + 65], 1.0)

                def qkv_nat_main(m):
                    ps_q = psA.tile([P, HCOLS], F32, name=f"ps_q{m % 2}",
                                    bufs=1)
                    for k in range(KS):
                        nc.tensor.matmul(
                            ps_q[:], xtt[:, k, m * P:(m + 1) * P], wqt[:, k],
                            start=(k == 0), stop=False,
                        )
                    return ps_q

                def qkv_nat_fin(m, ps_q):
                    nc.tensor.matmul(
                        ps_q[:], xtt[:, KS, m * P:(m + 1) * P], wqt[:, KS],
                        start=False, stop=True,
                    )
                    nc.vector.tensor_scalar_mul(
                        qkv[:, m, :].rearrange(
                            "p (h c) -> p h c", c=68
                        )[:, :, 0:64],
                        ps_q[:].rearrange("p (h c) -> p h c", c=64),
                        rsq_c[:, m:m + 1],
                    )

                early = {m: qkv_nat_main(m) for m in range(2)}

                # broadcast r1 across partitions via K=1 matmul
                ps_r1 = psA.tile([P, S], F32, name="ps_r1", tag="ps_a")
                for nb in range(2):
                    qs = slice(512 * nb, 512 * nb + 512)
                    nc.tensor.matmul(
                        ps_r1[:, qs], ones1t[:], r1row[:, qs],
                        start=True, stop=True,
                    )
                R1 = pha.tile([P, S], BF16, tag="bcastbuf", bufs=2)
                nc.vector.tensor_copy(R1[:], ps_r1[:])

                # close the strip groups with the aug subtile, then evac
                # qkvT = psum * r1 (accum_out gives colsum_v for free)
                for m in range(HM):
                    for nb in range(2):
                        qs = slice(512 * nb, 512 * nb + 512)
                        nc.tensor.matmul(
                            ps_qts[m][:, qs],
                            wqt[:, KS, m * P:(m + 1) * P],
                            xtt[:, KS, qs],
                            start=False, stop=True,
                        )
                    nc.vector.scalar_tensor_tensor(
                        out=qkvT[:, m, :], in0=ps_qts[m][:], scalar=1.0,
                        in1=R1[:], op0=MULT, op1=MULT,
                        accum_out=csum[:, m:m + 1],
                    )

                # colsum_v -> row layout -> caugt head segments
                nc.scalar.dma_start(
                    scr_cs[:].rearrange("t p -> p t"), csum[:]
                )
                nc.scalar.dma_start(
                    caugt[0:1, :].rearrange(
                        "one (h c) -> one h c", c=68
                    )[:, :, 0:64],
                    scr_cs[:].rearrange("t p -> (t p)").unsqueeze(0)
                    .rearrange("one (h c) -> one h c", c=64),
                )

                for m in range(2):
                    qkv_nat_fin(m, early[m])
                for m in range(2, KS):
                    ps_q = qkv_nat_main(m)
                    qkv_nat_fin(m, ps_q)

                psA_cm.__exit__(None, None, None)
                pha2_cm.__exit__(None, None, None)

                # identity for phase-D transposes, built BEFORE the
                # collectives are issued (gpsimd queue would otherwise
                # stall it behind the ReduceScatters)
                ident = glob.tile([P, P], BF16)
                make_identity(nc, ident[:])

                # ============= Phase B/C: attention pipelined by q-half ====
                with (
                    tc.tile_pool(name="phb", bufs=1) as phb,
                    tc.tile_pool(name="psB", bufs=1, space="PSUM") as psB,
                ):
                    for qh in range(2):
                        qsl = slice(512 * qh, 512 * qh + 512)
                        maskq = maskA if qh == 0 else maskB2
                        for pair in range(2):
                            # two m-subtiles per exp/fixup op; ctx matmuls
                            # interleave with scores so the PE never idles
                            # long enough for HAM to re-throttle
                            pcts = {}
                            for h in (2 * pair, 2 * pair + 1):
                                pcts[h] = psB.tile(
                                    [P, 512], F32, name=f"ps_ct{h % 2}",
                                    bufs=1,
                                )
                            for mm2 in range(4):
                                eTms = {}
                                for h in (2 * pair, 2 * pair + 1):
                                    hp = 64 * (h % 2)
                                    hs = h // 2
                                    ps_sc = psB.tile(
                                        [P, 1024], F32, name="ps_sc", bufs=2
                                    )
                                    for half in range(2):
                                        m = 2 * mm2 + half
                                        nc.tensor.matmul(
                                            ps_sc[:, 512 * half:
                                                  512 * half + 512],
                                            qkvT[hp:hp + 64, hs,
                                                 m * P:(m + 1) * P],
                                            qkvT[hp:hp + 64, hs, qsl],
                                            start=True, stop=True,
                                        )
                                    eraw = phb.tile(
                                        [P, 1024], BF16, name="eraw", bufs=4
                                    )
                                    nc.scalar.activation(
                                        eraw[:], ps_sc[:], AF.Exp,
                                        scale=float(1.0 / np.sqrt(DK)),
                                    )
                                    eTm = phb.tile(
                                        [P, 1024], BF16, name="eTm", bufs=4
                                    )
                                    eTms[h] = eTm
                                    mk = maskq[:, 2 * mm2:2 * mm2 + 2, :]
                                    em1 = phb.tile(
                                        [P, 1024], BF16, name="em1", bufs=2
                                    )
                                    nc.vector.tensor_scalar_sub(
                                        em1[:], eraw[:], 1.0
                                    )
                                    nc.vector.tensor_tensor(
                                        eTm[:], em1[:], mk, MULT
                                    )
                                for h in (2 * pair, 2 * pair + 1):
                                    for half in range(2):
                                        nc.tensor.matmul(
                                            pcts[h][0:65, :],
                                            qkv[:, 2 * mm2 + half,
                                                68 * h:68 * h + 65],
                                            eTms[h][:, 512 * half:
                                                    512 * half + 512],
                                            start=(mm2 == 0 and half == 0),
                                            stop=False,
                                        )
                            zhs = {}
                            for h in (2 * pair, 2 * pair + 1):
                                nc.tensor.matmul(
                                    pcts[h][0:65, :],
                                    caugt[0:1, 68 * h:68 * h + 65],
                                    ones_s[0:1, qsl],
                                    start=False, stop=True,
                                )
                                zh = phb.tile([1, 512], F32,
                                              name=f"zh{h % 2}",
                                              tag="zh", bufs=4)
                                zhs[h] = zh
                                nc.vector.tensor_copy(
                                    zh[:], pcts[h][64:65, :]
                                )
                            ps_nb = psB.tile([P, 1024], F32, name="ps_sc",
                                             bufs=2)
                            for h in (2 * pair, 2 * pair + 1):
                                hp = 64 * (h % 2)
                                nc.tensor.matmul(
                                    ps_nb[hp:hp + 64, 0:512],
                                    ones1t[0:1, 0:64], zhs[h][:],
                                    start=True, stop=True,
                                )
                            nrmb = pha.tile([P, 512], F32,
                                            name=f"nrmb{pair}_{qh}",
                                            tag="bcastbuf", bufs=2)
                            nc.vector.reciprocal(nrmb[:], ps_nb[:, 0:512])
                            for h in (2 * pair, 2 * pair + 1):
                                hp = 64 * (h % 2)
                                nc.vector.scalar_tensor_tensor(
                                    out=ctxn[hp:hp + 64, pair, qsl],
                                    in0=pcts[h][0:64, :], scalar=1.0,
                                    in1=nrmb[hp:hp + 64, :],
                                    op0=MULT, op1=MULT,
                                )
                        # attn-out partial for this q-half, then its RS
                        bnc = (bounce_inA if qh == 0 else bounce_inB)[
                            :
                        ].rearrange("(r p) d -> p r d", p=P)
                        for mt in range(4 * qh, 4 * qh + 4):
                            ps_ao = psB.tile([P, D], F32, name="ps_ao",
                                             bufs=1)
                            for nb in range(2):
                                ds_ = slice(512 * nb, 512 * nb + 512)
                                for k in range(HM):
                                    nc.tensor.matmul(
                                        ps_ao[:, ds_],
                                        ctxn[:, k, mt * P:(mt + 1) * P],
                                        wot[:, k, ds_],
                                        start=(k == 0), stop=(k == HM - 1),
                                    )
                            aot = phb.tile([P, D], BF16, tag="aot", bufs=2)
                            nc.scalar.copy(aot[:], ps_ao[:])
                            nc.sync.dma_start(bnc[:, mt - 4 * qh, :],
                                              aot[:])

            nc.gpsimd.collective_compute(
                "ReduceScatter",
                mybir.AluOpType.add,
                replica_groups=GROUPS,
                ins=[bounce_inA.opt()],
                outs=[bounce_rsA.opt()],
            )
            nc.gpsimd.collective_compute(
                "ReduceScatter",
                mybir.AluOpType.add,
                replica_groups=GROUPS,
                ins=[bounce_inB.opt()],
                outs=[bounce_rsB.opt()],
            )

            # ========== Phase D: residual + LN2 + FFN ==========
            with (
                tc.tile_pool(name="phd", bufs=1) as phd,
                tc.tile_pool(name="psD", bufs=1, space="PSUM") as psD,
            ):
                x2s = [None, None]
                n2s = [None, None]
                n2Ts = [None, None]

                def ln2_block(m):
                    aors = phd.tile([P, D], BF16, name=f"aors{m}")
                    nc.sync.dma_start(
                        aors[:], (bounce_rsA if m == 0 else bounce_rsB)[:]
                    )
                    x2 = phd.tile([P, D], F32, name=f"x2_{m}")
                    nc.vector.tensor_tensor(x2[:], aors[:], xslf[:, m], ADD)

                    s1 = phd.tile([P, 1], F32, name=f"s1_{m}")
                    s2 = phd.tile([P, 1], F32, name=f"s2_{m}")
                    sq2 = phd.tile([P, D], F32, tag="sq2", bufs=1)
                    nc.vector.reduce_sum(out=s1[:], in_=x2[:], axis=AX)
                    nc.scalar.activation(
                        sq2[:], x2[:], AF.Square, accum_out=s2[:]
                    )
                    mu2 = phd.tile([P, 1], F32, name=f"mu2_{m}")
                    nc.vector.tensor_scalar_mul(mu2[:], s1[:], 1.0 / D)
                    v2 = phd.tile([P, 1], F32, name=f"v2_{m}")
                    nc.vector.tensor_tensor(v2[:], s1[:], mu2[:], MULT)
                    nc.vector.tensor_tensor(v2[:], s2[:], v2[:], SUB)
                    nc.vector.tensor_scalar_mul(v2[:], v2[:], 1.0 / (D - 1))
                    std2 = phd.tile([P, 1], F32, name=f"std2_{m}")
                    nc.scalar.activation(std2[:], v2[:], AF.Sqrt)
                    nc.vector.tensor_scalar_add(std2[:], std2[:], EPS)
                    r2 = phd.tile([P, 1], F32, name=f"r2_{m}")
                    nc.vector.reciprocal(r2[:], std2[:])
                    n2 = phd.tile([P, D], BF16, name=f"n2_{m}")
                    nc.vector.scalar_tensor_tensor(
                        out=n2[:], in0=x2[:], scalar=mu2[:], op0=SUB,
                        in1=r2[:].to_broadcast((P, D)), op1=MULT,
                    )
                    x2s[m] = x2
                    n2s[m] = n2

                def transp_block(m):
                    n2T = phd.tile([P, KS, P], BF16, name=f"n2T{m}")
                    for i in range(KS):
                        ps_t = psD.tile([P, P], BF16, name="ps_t", bufs=2)
                        nc.tensor.transpose(
                            ps_t[:], n2s[m][:, i * P:(i + 1) * P], ident[:]
                        )
                        nc.vector.tensor_copy(n2T[:, i, :], ps_t[:])
                    n2Ts[m] = n2T

                def ff1_block(m, bias1t, reluT):
                    for g in range(8):
                        for i4 in range(4):
                            i = 4 * g + i4
                            ps_f = psD.tile([P, P], F32, name="ps_f", bufs=2)
                            for k in range(KS):
                                nc.tensor.matmul(
                                    ps_f[:],
                                    w1f[:, k, 128 * i:128 * i + 128],
                                    n2Ts[m][:, k, :],
                                    start=(k == 0), stop=(k == KS - 1),
                                )
                            nc.scalar.activation(
                                reluT[:, i, m * P:(m + 1) * P], ps_f[:],
                                AF.Relu, bias=bias1t[:, i:i + 1],
                            )

                # m=0 chain first (overlaps the second ReduceScatter);
                # every DMA that m=0 needs is issued before the aors1 DMA
                # so the RS-B wait cannot head-of-line-block them.
                ps_o0 = psD.tile([P, D], F32, name="ps_o0")
                ps_o1 = psD.tile([P, D], F32, name="ps_o1")
                ps_os = [ps_o0, ps_o1]

                def ff2_block(m, reluT):
                    for k in range(FFA):
                        for nb in range(2):
                            ds_ = slice(512 * nb, 512 * nb + 512)
                            nc.tensor.matmul(
                                ps_os[m][:, ds_],
                                reluT[:, k, m * P:(m + 1) * P],
                                w2f[:, k, ds_],
                                start=(k == 0), stop=(k == FFA - 1),
                            )
                    outt = phd.tile([P, D], F32, name=f"outt{m}")
                    nc.vector.tensor_tensor(
                        outt[:], ps_os[m][:], x2s[m][:], ADD
                    )
                    nc.scalar.dma_start(
                        out[:].rearrange("(m p) d -> p m d", p=P)[:, m, :],
                        outt[:],
                    )

                ln2_block(0)
                # memsets on DVE: the gpsimd queue is blocked behind the
                # collective triggers at this point in the program
                reluT = phd.tile([P, FFA, TS], BF16)
                nc.vector.memset(reluT[:, FFS, :], 0.0)
                nc.vector.memset(reluT[0:1, FFS, :], 1.0)
                transp_block(0)
                ff1_block(0, bias1t, reluT)
                ff2_block(0, reluT)
                ln2_block(1)
                transp_block(1)
                ff1_block(1, bias1t, reluT)
                ff2_block(1, reluT)
    nc.compile()
    return nc


def _prep_inputs(x, mask, Wq, Wo, W1, B1, W2, B2, ln1_a, ln1_b, ln2_a, ln2_b):
    """Host-side folding + striping. Returns in_maps for 8 cores."""
    f32 = np.float32

    def strip(a, ks):  # [ks*128, F] -> [128, ks, F]
        return np.ascontiguousarray(
            a.reshape(ks, P, -1).transpose(1, 0, 2).astype(f32)
        )

    Wa = (Wq * ln1_a[:, None]).astype(f32)          # LN1 scale folded
    g = Wa.sum(axis=0)                               # [D]
    c1 = (Wq.T @ ln1_b).astype(f32)                  # [D]
    Wa1 = (W1 * ln2_a[:, None]).astype(f32)
    bias1_full = (B1 + W1.T @ ln2_b).astype(f32)     # [DFF]

    w1_s = strip(Wa1, KS).astype(ml_dtypes.bfloat16)  # [128, 8, 4096]
    w2_aug = np.zeros((FFA * P, D), f32)
    w2_aug[:DFF] = W2
    w2_aug[DFF] = B2
    w2_s = strip(w2_aug, FFA).astype(ml_dtypes.bfloat16)  # [128, 33, 1024]
    bias1_s = np.ascontiguousarray(bias1_full.reshape(FFS, P).T)  # [128, 32]

    ones1 = np.ones((1, P), f32)
    caug_h = np.zeros((1, 68 * HD), ml_dtypes.bfloat16)
    for h in range(HD):
        caug_h[0, 68 * h + 64] = float(S)

    in_maps = []
    for c in range(NC):
        b, j = divmod(c, 4)
        cols = slice(HCOLS * j, HCOLS * j + HCOLS)
        tok_blocks = [j, j + 4]  # 128-token blocks owned by this core

        xt_aug = np.zeros((P, KA, S), ml_dtypes.bfloat16)
        xt_aug[:, :KS] = strip(np.ascontiguousarray(x[b].T), KS)
        xt_aug[1, KS] = 1.0
        xnat_s = strip(np.asarray(x[b], f32), SM).astype(ml_dtypes.bfloat16)
        wq_aug = np.zeros((P, KA, HCOLS), ml_dtypes.bfloat16)
        wq_aug[:, :KS] = strip(Wa[:, cols], KS)
        wq_aug[0, KS] = g[cols]
        wq_aug[1, KS] = c1[cols]
        maskt_ = np.ascontiguousarray(mask[b, 0].T).astype(f32)

        in_maps.append({
            "xt": xt_aug,
            "xnat": xnat_s,
            "wq": wq_aug,
            "ones1": ones1,
            "caug": caug_h,
            "xsl": np.ascontiguousarray(np.stack(
                [x[b, 128 * t:128 * t + 128] for t in tok_blocks], axis=1
            ).transpose(0, 1, 2)).reshape(P, TM, D).astype(ml_dtypes.bfloat16),
            "maskt": strip(maskt_, KS).astype(ml_dtypes.bfloat16),
            "wo": strip(
                np.ascontiguousarray(np.asarray(Wo, f32)[cols]), HM
            ).astype(ml_dtypes.bfloat16),
            "w1": w1_s,
            "w2": w2_s,
            "bias1": bias1_s,
        })
    return in_maps


def kernel(**inputs):
    if "nc" not in _CACHE:
        _CACHE["nc"] = _build()
    nc = _CACHE["nc"]
    args = {k: np.asarray(v) for k, v in inputs.items()}
    in_maps = _prep_inputs(
        args["x"], args["mask"], args["Wq"], args["Wo"], args["W1"],
        args["B1"], args["W2"], args["B2"], args["ln1_a"], args["ln1_b"],
        args["ln2_a"], args["ln2_b"],
    )
    res = bass_utils.run_bass_kernel_spmd(
        nc, in_maps, core_ids=list(range(NC))
    )
    out = np.empty((B, S, D), np.float32)
    for c in range(NC):
        b, j = divmod(c, 4)
        o = res.results[c]["out"]
        out[b, 128 * j:128 * j + 128] = o[0:128]
        out[b, 512 + 128 * j:512 + 128 * j + 128] = o[128:256]
    return out



# revision 47
# speedup vs baseline: 1.0527x; 1.0331x over previous
"""Trainium2 Bass kernel for nn_EncoderBlock (pre-norm self-attention + FFN).

Sharding (8 cores): core c -> batch b = c//4, head-group j = c%4
(heads 4j..4j+3, Wq columns 256j..256j+256, Wo rows 256j..256j+256),
token slice 256j..256j+256 for the post-attention residual + FFN part.
One ReduceScatter per q-half (the "all-reduce after Wo" of the TP hint,
fused with the token scatter) inside each 4-core batch group.

Key tricks (v2):
 - LN1 folded into the QKV matmuls via TWO augmented contraction rows
   (row0: -mu x g, row1: ones x c1 = Wq^T ln1_b), with the 1/std scale
   fused into the PSUM evacuation (single scalar_tensor_tensor op whose
   accum_out also yields colsum_v for free).
 - LN1 stats computed from a natural-layout copy of x on ACT (Square +
   accum_out) and GpSimd (free-axis reduce) -- zero PE work, overlapped
   with the QKV matmuls; 1/(std+eps) ~= Rsqrt(var) (error ~1e-6).
 - masked_fill(mask==0, 1e-9): exp runs directly on the scores PSUM
   (ACT, scale=1/8), then one fused bf16 op f = (e-1)*m at 2x DVE rate.
   The "masked entries contribute exactly 1" correction is a rank-1
   augmented row on the ctx matmul: ctx += colsum_v (x) ones, Z += S.
 - Softmax Z via an appended ones column (PSUM row 64); normalization
   uses a K=4 selector matmul to broadcast Z across partitions.
 - FFN token-sharded with full weights; W1 is prefetched once into SBUF
   (gpsimd DMA queue) and reused for both token blocks; ff1/relu for
   block 0 overlaps the second ReduceScatter; W2 streamed once.

All big matmuls run in bf16/float32r.
"""

import numpy as np
import ml_dtypes

import concourse.bass as bass
import concourse.mybir as mybir
import concourse.tile as tile
from concourse import bacc
from concourse import bass_utils
from concourse.masks import make_identity

F32 = mybir.dt.float32
F32R = mybir.dt.float32r
BF16 = mybir.dt.bfloat16
AF = mybir.ActivationFunctionType
MULT = mybir.AluOpType.mult
ADD = mybir.AluOpType.add
SUB = mybir.AluOpType.subtract
AX = mybir.AxisListType.X

B, S, D, H, DK, DFF = 2, 1024, 1024, 16, 64, 4096
EPS = 1e-6
P = 128
NC = 8
KS = D // P            # 8 k-subtiles over d_model
KA = KS + 1            # + augmented subtile (row0 = -mu, row1 = ones)
FFS = DFF // P         # 32 ff subtiles
FFA = FFS + 1          # + augmented subtile (row 0 = ones -> B2)
TS = S // 4            # 256-token slice per core
TM = TS // P           # 2 token M-tiles
SM = S // P            # 8 token M-tiles (full sequence)
HD = 4                 # heads per core
HCOLS = HD * DK        # 256 qkv columns per core
HM = HCOLS // P        # 2 partition strips of qkv columns
GROUPS = [[0, 1, 2, 3], [4, 5, 6, 7]]

_CACHE = {}


def _build():
    nc = bacc.Bacc("TRN2", target_bir_lowering=False, debug=False, num_devices=NC)

    def din(name, shape, dt):
        return nc.dram_tensor(name, shape, dt, kind="ExternalInput")

    xt = din("xt", [P, KA, S], BF16)          # x[b]^T striped + aug subtile
    xnat = din("xnat", [P, SM, D], BF16)      # x[b] natural, token-striped
    wq = din("wq", [P, KA, HCOLS], BF16)      # (Wq*a1) cols; aug row0=g row1=c1
    ones1 = din("ones1", [1, P], F32)        # ones row for partition-bcast
    sel4 = din("sel4", [HD, HCOLS], F32R)     # head-selector for Z broadcast
    caug = din("caug", [1, 68 * HD], F32)    # ctx aug lhsT: col 68h+64 = S
    xsl = din("xsl", [P, TM, D], F32)         # x token-slice (natural)
    maskt = din("maskt", [P, KS, S], BF16)    # mask[b,0]^T striped
    wo = din("wo", [P, HM, D], BF16)          # Wo rows 256j..256j+256, striped
    w1 = din("w1", [P, KS, DFF], BF16)        # W1*a2 striped (lhsT)
    w2 = din("w2", [P, FFA, D], BF16)         # [W2 ; B2 ; 0-pad] striped (rhs)
    bias1 = din("bias1", [P, FFS], F32)       # B1 + W1^T ln2_b, striped cols
    fftail = din("fftail", [P, TS], BF16)     # relu aug tail: row0=ones
    out = nc.dram_tensor("out", [TS, D], F32, kind="ExternalOutput")

    with tile.TileContext(nc) as tc:
        with (
            tc.tile_pool(name="glob", bufs=1) as glob,
            tc.tile_pool(name="gdram", bufs=1, space="DRAM") as gdram,
        ):
            # ---- tiles that cross phases ----
            qkvT = glob.tile([P, HM, S], BF16)           # [d'(2 strips), tok]
            qkv = glob.tile([P, KS, 68 * HD], BF16)      # per-head 64+ones+3pad
            ctxn = glob.tile([P, HM, S], BF16)           # normalized ctx^T
            csum = glob.tile([P, HM], F32)               # colsum of qkvT strips
            w1f = glob.tile([P, KS, DFF], BF16)          # full W1, SBUF-resident
            ones1t = glob.tile([1, P], F32)
            nc.sync.dma_start(ones1t[:], ones1[:])
            sel4t = glob.tile([HD, HCOLS], F32R)
            nc.sync.dma_start(sel4t[:], sel4[:])
            ones_s = glob.tile([1, S], F32)
            nc.gpsimd.memset(ones_s[:], 1.0)
            caugt = glob.tile([1, 68 * HD], F32)
            nc.sync.dma_start(caugt[:], caug[:])
            scr_mu = gdram.tile([SM, P], BF16)           # -mu round-trip
            scr_rs = gdram.tile([SM, P], F32)           # rsqrt(var) round-trip
            scr_cs = gdram.tile([HM, P], F32)           # colsum_v round-trip
            bounce_inA = gdram.tile([S // 2, D], BF16)   # attn-out partial, even
            bounce_inB = gdram.tile([S // 2, D], BF16)   # attn-out partial, odd
            bounce_rsA = gdram.tile([P, D], BF16)        # my tokens 0:128, summed
            bounce_rsB = gdram.tile([P, D], BF16)        # my tokens 128:256

            # ================= Phase A: LN1 stats + QKV =================
            with tc.tile_pool(name="pha", bufs=1) as pha:
                pha2_cm = tc.tile_pool(name="pha2", bufs=1)
                pha2 = pha2_cm.__enter__()
                psA_cm = tc.tile_pool(name="psA", bufs=1, space="PSUM")
                psA = psA_cm.__enter__()
                xtt = pha2.tile([P, KA, S], BF16)
                for k in range(KA):
                    nc.sync.dma_start(xtt[:, k], xt[:, k])
                wqt = pha2.tile([P, KA, HCOLS], BF16)
                nc.sync.dma_start(wqt[:], wq[:])
                masktt = pha.tile([P, KS, S], BF16)
                nc.sync.dma_start(masktt[:], maskt[:])
                wot = pha.tile([P, HM, D], BF16)
                nc.sync.dma_start(wot[:], wo[:])
                xnt = pha2.tile([P, SM, D], BF16)
                nc.scalar.dma_start(xnt[:], xnat[:])

                # LN1 stats: S2 on ACT (Square + accum), S1 on GpSimd
                s1c = pha2.tile([P, SM], F32)
                s2c = pha2.tile([P, SM], F32)
                sqscr = pha2.tile([P, D], BF16, tag="sqscr", bufs=2)
                cpscr = pha2.tile([P, D], BF16, tag="cpscr", bufs=2)
                for t in range(SM):
                    nc.scalar.activation(
                        sqscr[:], xnt[:, t], AF.Square,
                        accum_out=s2c[:, t:t + 1],
                    )
                    nc.vector.tensor_scalar(
                        out=cpscr[:], in0=xnt[:, t], scalar1=1.0,
                        scalar2=0.0, op0=MULT, op1=ADD,
                        accum_out=s1c[:, t:t + 1],
                    )
                # column math on [P, SM]: mu, var, rsqrt(var)
                mu_c = pha2.tile([P, SM], F32)
                nc.vector.tensor_scalar_mul(mu_c[:], s1c[:], 1.0 / D)
                tv = pha2.tile([P, SM], F32)
                nc.vector.tensor_tensor(tv[:], s1c[:], mu_c[:], MULT)
                nc.vector.tensor_tensor(tv[:], s2c[:], tv[:], SUB)
                nc.vector.tensor_scalar_mul(tv[:], tv[:], 1.0 / (D - 1))
                stdc = pha2.tile([P, SM], F32)
                nc.scalar.activation(stdc[:], tv[:], AF.Sqrt)
                nc.vector.tensor_scalar_add(stdc[:], stdc[:], EPS)
                rsq_c = pha2.tile([P, SM], F32)           # = r1col (recip std)
                nc.vector.reciprocal(rsq_c[:], stdc[:])
                mu_n = pha2.tile([P, SM], BF16)
                nc.vector.tensor_scalar_mul(mu_n[:], s1c[:], -1.0 / D)

                # round-trips to row layout: -mu -> xt aug row0, rsq -> r1row
                nc.sync.dma_start(
                    scr_mu[:].rearrange("t p -> p t"), mu_n[:]
                )
                nc.sync.dma_start(
                    xtt[0:1, KS, :],
                    scr_mu[:].rearrange("t p -> (t p)").unsqueeze(0),
                )
                nc.sync.dma_start(
                    scr_rs[:].rearrange("t p -> p t"), rsq_c[:]
                )
                r1row = pha2.tile([1, S], F32)
                nc.sync.dma_start(
                    r1row[:],
                    scr_rs[:].rearrange("t p -> (t p)").unsqueeze(0),
                )

                # PE program order: the k<=7 accumulations of both qkvT
                # strips and the first two qkv-natural tiles gate only on
                # xtt/wqt, so the PE starts ~6us in; the r1 broadcast, aug
                # matmuls and evacs (gated on the stats chain) come after.
                ps_qts = [
                    psA.tile([P, S], F32, name=f"ps_qt{m}", bufs=1)
                    for m in range(HM)
                ]
                for m in range(HM):
                    for nb in range(2):
                        qs = slice(512 * nb, 512 * nb + 512)
                        for k in range(KS):
                            nc.tensor.matmul(
                                ps_qts[m][:, qs],
                                wqt[:, k, m * P:(m + 1) * P],
                                xtt[:, k, qs],
                                start=(k == 0), stop=False,
                            )
                for h in range(HD):
                    nc.gpsimd.memset(qkv[:, :, 68 * h + 64:68 * h + 65], 1.0)

                def qkv_nat_main(m):
                    ps_q = psA.tile([P, HCOLS], F32, name=f"ps_q{m % 2}",
                                    bufs=1)
                    for k in range(KS):
                        nc.tensor.matmul(
                            ps_q[:], xtt[:, k, m * P:(m + 1) * P], wqt[:, k],
                            start=(k == 0), stop=False,
                        )
                    return ps_q

                def qkv_nat_fin(m, ps_q):
                    nc.tensor.matmul(
                        ps_q[:], xtt[:, KS, m * P:(m + 1) * P], wqt[:, KS],
                        start=False, stop=True,
                    )
                    nc.vector.tensor_scalar_mul(
                        qkv[:, m, :].rearrange(
                            "p (h c) -> p h c", c=68
                        )[:, :, 0:64],
                        ps_q[:].rearrange("p (h c) -> p h c", c=64),
                        rsq_c[:, m:m + 1],
                    )

                early = {m: qkv_nat_main(m) for m in range(2)}

                # broadcast r1 across partitions via K=1 matmul
                ps_r1 = psA.tile([P, S], F32, name="ps_r1", tag="ps_a")
                for nb in range(2):
                    qs = slice(512 * nb, 512 * nb + 512)
                    nc.tensor.matmul(
                        ps_r1[:, qs], ones1t[:], r1row[:, qs],
                        start=True, stop=True,
                    )
                R1 = pha.tile([P, S], F32, tag="bcastbuf", bufs=2)
                nc.vector.tensor_copy(R1[:], ps_r1[:])
                # W1 prefetch: one contiguous transfer (64KB/partition),
                # gated on the last phase-A input DMA landing so its 64KB
                # packets cannot starve the small input packets.
                wtiny = pha2.tile([1, 1], BF16)
                nc.scalar.copy(wtiny[:], xnt[0:1, 0, 0:1])
                nc.scalar.dma_start(w1f[:], w1[:])

                # close the strip groups with the aug subtile, then evac
                # qkvT = psum * r1 (accum_out gives colsum_v for free)
                for m in range(HM):
                    for nb in range(2):
                        qs = slice(512 * nb, 512 * nb + 512)
                        nc.tensor.matmul(
                            ps_qts[m][:, qs],
                            wqt[:, KS, m * P:(m + 1) * P],
                            xtt[:, KS, qs],
                            start=False, stop=True,
                        )
                    nc.vector.scalar_tensor_tensor(
                        out=qkvT[:, m, :], in0=ps_qts[m][:], scalar=1.0,
                        in1=R1[:], op0=MULT, op1=MULT,
                        accum_out=csum[:, m:m + 1],
                    )

                # colsum_v -> row layout -> caugt head segments
                nc.sync.dma_start(
                    scr_cs[:].rearrange("t p -> p t"), csum[:]
                )
                nc.sync.dma_start(
                    caugt[0:1, :].rearrange(
                        "one (h c) -> one h c", c=68
                    )[:, :, 0:64],
                    scr_cs[:].rearrange("t p -> (t p)").unsqueeze(0)
                    .rearrange("one (h c) -> one h c", c=64),
                )

                for m in range(2):
                    qkv_nat_fin(m, early[m])
                for m in range(2, KS):
                    ps_q = qkv_nat_main(m)
                    qkv_nat_fin(m, ps_q)

                psA_cm.__exit__(None, None, None)
                pha2_cm.__exit__(None, None, None)

                # identity for phase-D transposes, built BEFORE the
                # collectives are issued (gpsimd queue would otherwise
                # stall it behind the ReduceScatters)
                ident = glob.tile([P, P], BF16)
                make_identity(nc, ident[:])

                # ============= Phase B/C: attention pipelined by q-half ====
                with (
                    tc.tile_pool(name="phb", bufs=1) as phb,
                    tc.tile_pool(name="psB", bufs=1, space="PSUM") as psB,
                ):
                    aosb = pha.tile([P, SM, D], BF16, tag="bigbuf")
                    for qh in range(2):
                        qsl = slice(512 * qh, 512 * qh + 512)
                        for pair in range(2):
                            # two m-subtiles per exp/fixup op; ctx matmuls
                            # interleave with scores so the PE never idles
                            # long enough for HAM to re-throttle
                            pcts = {}
                            for h in (2 * pair, 2 * pair + 1):
                                pcts[h] = psB.tile(
                                    [P, 512], F32, name=f"ps_ct{h % 2}",
                                    bufs=1,
                                )
                            for mm2 in range(4):
                                eTms = {}
                                for h in (2 * pair, 2 * pair + 1):
                                    hp = 64 * (h % 2)
                                    hs = h // 2
                                    ps_sc = psB.tile(
                                        [P, 1024], F32, name="ps_sc", bufs=2
                                    )
                                    for half in range(2):
                                        m = 2 * mm2 + half
                                        nc.tensor.matmul(
                                            ps_sc[:, 512 * half:
                                                  512 * half + 512],
                                            qkvT[hp:hp + 64, hs,
                                                 m * P:(m + 1) * P],
                                            qkvT[hp:hp + 64, hs, qsl],
                                            start=True, stop=True,
                                        )
                                    eraw = phb.tile(
                                        [P, 1024], BF16, name="eraw", bufs=4
                                    )
                                    nc.scalar.activation(
                                        eraw[:], ps_sc[:], AF.Exp,
                                        scale=float(1.0 / np.sqrt(DK)),
                                    )
                                    eTm = phb.tile(
                                        [P, 1024], BF16, name="eTm", bufs=4
                                    )
                                    eTms[h] = eTm
                                    mk = masktt[:, 2 * mm2:2 * mm2 + 2, qsl]
                                    em1 = phb.tile(
                                        [P, 1024], BF16, name="em1", bufs=4
                                    )
                                    nc.vector.tensor_scalar_sub(
                                        em1[:], eraw[:], 1.0
                                    )
                                    nc.vector.tensor_tensor(
                                        eTm[:], em1[:], mk, MULT
                                    )
                                for h in (2 * pair, 2 * pair + 1):
                                    for half in range(2):
                                        nc.tensor.matmul(
                                            pcts[h][0:65, :],
                                            qkv[:, 2 * mm2 + half,
                                                68 * h:68 * h + 65],
                                            eTms[h][:, 512 * half:
                                                    512 * half + 512],
                                            start=(mm2 == 0 and half == 0),
                                            stop=False,
                                        )
                            zhs = {}
                            for h in (2 * pair, 2 * pair + 1):
                                nc.tensor.matmul(
                                    pcts[h][0:65, :],
                                    caugt[0:1, 68 * h:68 * h + 65],
                                    ones_s[0:1, qsl],
                                    start=False, stop=True,
                                )
                                zh = phb.tile([1, 512], F32,
                                              name=f"zh{h % 2}",
                                              tag="zh", bufs=4)
                                zhs[h] = zh
                                nc.vector.tensor_copy(
                                    zh[:], pcts[h][64:65, :]
                                )
                            ps_nb = psB.tile([P, 1024], F32, name="ps_sc",
                                             bufs=2)
                            for h in (2 * pair, 2 * pair + 1):
                                hp = 64 * (h % 2)
                                nc.tensor.matmul(
                                    ps_nb[hp:hp + 64, 0:512],
                                    ones1t[0:1, 0:64], zhs[h][:],
                                    start=True, stop=True,
                                )
                            nrmb = pha.tile([P, 512], F32,
                                            name=f"nrmb{pair}_{qh}",
                                            tag="bcastbuf", bufs=2)
                            nc.vector.reciprocal(nrmb[:], ps_nb[:, 0:512])
                            for h in (2 * pair, 2 * pair + 1):
                                hp = 64 * (h % 2)
                                nc.vector.scalar_tensor_tensor(
                                    out=ctxn[hp:hp + 64, pair, qsl],
                                    in0=pcts[h][0:64, :], scalar=1.0,
                                    in1=nrmb[hp:hp + 64, :],
                                    op0=MULT, op1=MULT,
                                )
                        # attn-out partial for this q-half, then its RS
                        for mt in range(4 * qh, 4 * qh + 4):
                            ps_ao = psB.tile([P, D], F32, name="ps_ao",
                                             bufs=1)
                            for nb in range(2):
                                ds_ = slice(512 * nb, 512 * nb + 512)
                                for k in range(HM):
                                    nc.tensor.matmul(
                                        ps_ao[:, ds_],
                                        ctxn[:, k, mt * P:(mt + 1) * P],
                                        wot[:, k, ds_],
                                        start=(k == 0), stop=(k == HM - 1),
                                    )
                            nc.scalar.copy(aosb[:, mt, :], ps_ao[:])
                        nc.sync.dma_start(
                            (bounce_inA if qh == 0 else bounce_inB)[
                                :
                            ].rearrange("(r p) d -> p r d", p=P),
                            aosb[:, 4 * qh:4 * qh + 4, :],
                        )

            nc.gpsimd.collective_compute(
                "ReduceScatter",
                mybir.AluOpType.add,
                replica_groups=GROUPS,
                ins=[bounce_inA.opt()],
                outs=[bounce_rsA.opt()],
            )
            nc.gpsimd.collective_compute(
                "ReduceScatter",
                mybir.AluOpType.add,
                replica_groups=GROUPS,
                ins=[bounce_inB.opt()],
                outs=[bounce_rsB.opt()],
            )

            # ========== Phase D: residual + LN2 + FFN ==========
            with (
                tc.tile_pool(name="phd", bufs=1) as phd,
                tc.tile_pool(name="psD", bufs=1, space="PSUM") as psD,
            ):
                x2s = [None, None]
                n2s = [None, None]
                n2Ts = [None, None]

                def ln2_block(m):
                    aors = phd.tile([P, D], BF16, name=f"aors{m}")
                    nc.sync.dma_start(
                        aors[:], (bounce_rsA if m == 0 else bounce_rsB)[:]
                    )
                    xslt = phd.tile([P, D], F32, name=f"xslt{m}")
                    nc.sync.dma_start(xslt[:], xsl[:, m])
                    x2 = phd.tile([P, D], F32, name=f"x2_{m}")
                    nc.vector.tensor_tensor(x2[:], aors[:], xslt[:], ADD)

                    s1 = phd.tile([P, 1], F32, name=f"s1_{m}")
                    s2 = phd.tile([P, 1], F32, name=f"s2_{m}")
                    sq2 = phd.tile([P, D], F32, tag="sq2", bufs=2)
                    nc.vector.reduce_sum(out=s1[:], in_=x2[:], axis=AX)
                    nc.scalar.activation(
                        sq2[:], x2[:], AF.Square, accum_out=s2[:]
                    )
                    mu2 = phd.tile([P, 1], F32, name=f"mu2_{m}")
                    nc.vector.tensor_scalar_mul(mu2[:], s1[:], 1.0 / D)
                    v2 = phd.tile([P, 1], F32, name=f"v2_{m}")
                    nc.vector.tensor_tensor(v2[:], s1[:], mu2[:], MULT)
                    nc.vector.tensor_tensor(v2[:], s2[:], v2[:], SUB)
                    nc.vector.tensor_scalar_mul(v2[:], v2[:], 1.0 / (D - 1))
                    std2 = phd.tile([P, 1], F32, name=f"std2_{m}")
                    nc.scalar.activation(std2[:], v2[:], AF.Sqrt)
                    nc.vector.tensor_scalar_add(std2[:], std2[:], EPS)
                    r2 = phd.tile([P, 1], F32, name=f"r2_{m}")
                    nc.vector.reciprocal(r2[:], std2[:])
                    n2 = phd.tile([P, D], BF16, name=f"n2_{m}")
                    nc.vector.scalar_tensor_tensor(
                        out=n2[:], in0=x2[:], scalar=mu2[:], op0=SUB,
                        in1=r2[:].to_broadcast((P, D)), op1=MULT,
                    )
                    x2s[m] = x2
                    n2s[m] = n2

                def transp_block(m):
                    n2T = phd.tile([P, KS, P], BF16, name=f"n2T{m}")
                    for i in range(KS):
                        ps_t = psD.tile([P, P], BF16, name="ps_t", bufs=2)
                        nc.tensor.transpose(
                            ps_t[:], n2s[m][:, i * P:(i + 1) * P], ident[:]
                        )
                        nc.vector.tensor_copy(n2T[:, i, :], ps_t[:])
                    n2Ts[m] = n2T

                def ff1_block(m, bias1t, reluT):
                    for g in range(8):
                        for i4 in range(4):
                            i = 4 * g + i4
                            ps_f = psD.tile([P, P], F32, name="ps_f", bufs=2)
                            for k in range(KS):
                                nc.tensor.matmul(
                                    ps_f[:],
                                    w1f[:, k, 128 * i:128 * i + 128],
                                    n2Ts[m][:, k, :],
                                    start=(k == 0), stop=(k == KS - 1),
                                )
                            nc.scalar.activation(
                                reluT[:, i, m * P:(m + 1) * P], ps_f[:],
                                AF.Relu, bias=bias1t[:, i:i + 1],
                            )

                # m=0 chain first (overlaps the second ReduceScatter);
                # every DMA that m=0 needs is issued before the aors1 DMA
                # so the RS-B wait cannot head-of-line-block them.
                ps_o0 = psD.tile([P, D], F32, name="ps_o0")
                ps_o1 = psD.tile([P, D], F32, name="ps_o1")
                ps_os = [ps_o0, ps_o1]

                def ff2_block(m, reluT):
                    # W2 streamed per-m (re-streamed for m=1) so that the
                    # m=0 pass can run inside the RS-B window
                    for k in range(FFA):
                        w2t = phd.tile([P, D], BF16, tag=f"w2t{m}", bufs=4)
                        nc.sync.dma_start(w2t[:], w2[:, k, :])
                        for nb in range(2):
                            ds_ = slice(512 * nb, 512 * nb + 512)
                            nc.tensor.matmul(
                                ps_os[m][:, ds_],
                                reluT[:, k, m * P:(m + 1) * P],
                                w2t[:, ds_],
                                start=(k == 0), stop=(k == FFA - 1),
                            )
                    outt = phd.tile([P, D], F32, name=f"outt{m}")
                    nc.vector.tensor_tensor(
                        outt[:], ps_os[m][:], x2s[m][:], ADD
                    )
                    nc.sync.dma_start(
                        out[:].rearrange("(m p) d -> p m d", p=P)[:, m, :],
                        outt[:],
                    )

                ln2_block(0)
                bias1t = phd.tile([P, FFS], F32)
                nc.sync.dma_start(bias1t[:], bias1[:])
                reluT = phd.tile([P, FFA, TS], BF16)
                nc.sync.dma_start(reluT[:, FFS, :], fftail[:])
                transp_block(0)
                ff1_block(0, bias1t, reluT)
                ff2_block(0, reluT)
                ln2_block(1)
                transp_block(1)
                ff1_block(1, bias1t, reluT)
                ff2_block(1, reluT)
    nc.compile()
    return nc


def _prep_inputs(x, mask, Wq, Wo, W1, B1, W2, B2, ln1_a, ln1_b, ln2_a, ln2_b):
    """Host-side folding + striping. Returns in_maps for 8 cores."""
    f32 = np.float32

    def strip(a, ks):  # [ks*128, F] -> [128, ks, F]
        return np.ascontiguousarray(
            a.reshape(ks, P, -1).transpose(1, 0, 2).astype(f32)
        )

    Wa = (Wq * ln1_a[:, None]).astype(f32)          # LN1 scale folded
    g = Wa.sum(axis=0)                               # [D]
    c1 = (Wq.T @ ln1_b).astype(f32)                  # [D]
    Wa1 = (W1 * ln2_a[:, None]).astype(f32)
    bias1_full = (B1 + W1.T @ ln2_b).astype(f32)     # [DFF]

    w1_s = strip(Wa1, KS).astype(ml_dtypes.bfloat16)  # [128, 8, 4096]
    w2_aug = np.zeros((FFA * P, D), f32)
    w2_aug[:DFF] = W2
    w2_aug[DFF] = B2
    w2_s = strip(w2_aug, FFA).astype(ml_dtypes.bfloat16)  # [128, 33, 1024]
    bias1_s = np.ascontiguousarray(bias1_full.reshape(FFS, P).T)  # [128, 32]

    ones1 = np.ones((1, P), f32)
    sel4 = np.zeros((HD, HCOLS), f32)
    for h in range(HD):
        sel4[h, 64 * h:64 * h + 64] = 1.0
    caug_h = np.zeros((1, 68 * HD), f32)
    for h in range(HD):
        caug_h[0, 68 * h + 64] = float(S)
    fftail = np.zeros((P, TS), ml_dtypes.bfloat16)
    fftail[0] = 1.0

    in_maps = []
    for c in range(NC):
        b, j = divmod(c, 4)
        cols = slice(HCOLS * j, HCOLS * j + HCOLS)
        tok_blocks = [j, j + 4]  # 128-token blocks owned by this core

        xt_aug = np.zeros((P, KA, S), ml_dtypes.bfloat16)
        xt_aug[:, :KS] = strip(np.ascontiguousarray(x[b].T), KS)
        xt_aug[1, KS] = 1.0
        xnat_s = strip(np.asarray(x[b], f32), SM).astype(ml_dtypes.bfloat16)
        wq_aug = np.zeros((P, KA, HCOLS), ml_dtypes.bfloat16)
        wq_aug[:, :KS] = strip(Wa[:, cols], KS)
        wq_aug[0, KS] = g[cols]
        wq_aug[1, KS] = c1[cols]
        maskt_ = np.ascontiguousarray(mask[b, 0].T).astype(f32)

        in_maps.append({
            "xt": xt_aug,
            "xnat": xnat_s,
            "wq": wq_aug,
            "ones1": ones1,
            "sel4": sel4,
            "caug": caug_h,
            "xsl": np.ascontiguousarray(np.stack(
                [x[b, 128 * t:128 * t + 128] for t in tok_blocks], axis=1
            ).transpose(0, 1, 2)).reshape(P, TM, D).astype(f32),
            "maskt": strip(maskt_, KS).astype(ml_dtypes.bfloat16),
            "wo": strip(
                np.ascontiguousarray(np.asarray(Wo, f32)[cols]), HM
            ).astype(ml_dtypes.bfloat16),
            "w1": w1_s,
            "w2": w2_s,
            "bias1": bias1_s,
            "fftail": fftail,
        })
    return in_maps


def kernel(**inputs):
    if "nc" not in _CACHE:
        _CACHE["nc"] = _build()
    nc = _CACHE["nc"]
    args = {k: np.asarray(v) for k, v in inputs.items()}
    in_maps = _prep_inputs(
        args["x"], args["mask"], args["Wq"], args["Wo"], args["W1"],
        args["B1"], args["W2"], args["B2"], args["ln1_a"], args["ln1_b"],
        args["ln2_a"], args["ln2_b"],
    )
    res = bass_utils.run_bass_kernel_spmd(
        nc, in_maps, core_ids=list(range(NC))
    )
    out = np.empty((B, S, D), np.float32)
    for c in range(NC):
        b, j = divmod(c, 4)
        o = res.results[c]["out"]
        out[b, 128 * j:128 * j + 128] = o[0:128]
        out[b, 512 + 128 * j:512 + 128 * j + 128] = o[128:256]
    return out
